# revision 24
# baseline (speedup 1.0000x reference)
"""LSTMCell (B=16384, IN=HID=512) on 8 TRN2 NeuronCores.

Strategy: data-parallel over batch (2048 rows/core), weights replicated.
Host pre-packs operands so the device kernel needs zero transposes:
  - GEMM computed as gates.T = W_cat.T @ [x;h].T  (K=1024 on partitions)
  - x/h/W cast to bf16 on host (fp32 PSUM accumulation on PE)
  - c / outputs stay fp32

v3 structure (baseline 135.7us -> v2 132.3us):
  - Weights repacked per output row-block r ([NR, P, NK, JW] DRAM layout,
    per-partition-contiguous lines).  r0's weights arrive as four 256KB
    quarter tiles so the very first matmul only needs 384KB of input;
    r1-r3 use 512KB half tiles.  All input DMAs ride the sync HW queue
    ordered by consumption deadline (bias first - it gates the first ACT).
  - nb0 runs k-outer/g-inner so weight chunks are consumed as they land;
    nb1-3 run g-outer/k-inner so the per-gate epilogue pipelines.
  - nb1-3 xh arrives as one 1MB DMA per nb (8KB/partition lines).
  - Output DMAs ride the gpsimd SW-DGE queue (never block inputs);
    the final nb's outputs ride sync (empty by then).
  - Last tile runs as two sequential N=256 PSUM groups so its epilogue
    overlaps the second half's matmuls; ACT(o) is queued before tanh.
  - 34 N=128 warmup matmuls keep the PE HAM busy-window alive from the
    first possible instruction (~7.6us) until data lands (~10.6us) so
    real matmuls run at 2.4GHz immediately.
"""

import sys

sys.path.insert(0, "/opt/trn_rl_repo")

from contextlib import ExitStack

import ml_dtypes
import numpy as np

import concourse.bass as bass  # noqa: F401  (bass types used via bacc/mybir)
import concourse.mybir as mybir
import concourse.tile as tile
from concourse import bacc
from concourse.bass_utils import run_bass_kernel_spmd

B_FULL, IN, HID = 16384, 512, 512
NCORES = 8
BL = B_FULL // NCORES  # 2048 batch rows per core
JW = 512               # batch columns per chunk (matmul free dim)
P = 128
H2 = JW // 2

BF16 = mybir.dt.bfloat16
F32 = mybir.dt.float32
AF = mybir.ActivationFunctionType
BF16_NP = ml_dtypes.bfloat16

NK = (IN + HID) // P   # 8  k-chunks of the contraction dim
NR = HID // P          # 4  row-blocks of H per gate
NM = 4 * HID // P      # 16 gate-row blocks total (i,g,f,o order)
NWU = 46               # warmup matmuls (N=128, ~107ns apiece cold); sized so
                       # they end ~12.1us, when the first input DMA semaphore
                       # fires (bytes land ~10.2us + ~1.7us completion latency)


def build_nc(bl=BL):
    """Build the single-core Bass program (SPMD-replicated across cores)."""
    nbn = bl // JW
    nc = bacc.Bacc("TRN2", target_bir_lowering=False, debug=False)

    # nb0's xh: k-major contiguous 128KB chunks (fine-grained startup);
    # nb>=1: partition-major 1MB blocks (8KB/partition lines, one DMA each).
    xh0_in = nc.dram_tensor("xh0_in", [NK, P, JW], BF16, kind="ExternalInput")
    if nbn > 1:
        xhb_in = nc.dram_tensor("xhb_in", [nbn - 1, P, NK, JW], BF16,
                                kind="ExternalInput")
    # r0 weights as 4 contiguous 256KB quarter chunks; r1-3 as contiguous
    # 512KB half chunks.
    wq_in = nc.dram_tensor("wq_in", [4, P, 2, JW], BF16, kind="ExternalInput")
    wh_in = nc.dram_tensor("wh_in", [NR - 1, 2, P, 4, JW], BF16,
                           kind="ExternalInput")
    bias_in = nc.dram_tensor("bias_in", [P, NM], F32, kind="ExternalInput")
    c_in = nc.dram_tensor("c_in", [nbn, NR, P, JW], F32, kind="ExternalInput")
    h_out = nc.dram_tensor("h_out", [nbn, NR, P, JW], F32, kind="ExternalOutput")
    c_out = nc.dram_tensor("c_out", [nbn, NR, P, JW], F32, kind="ExternalOutput")

    with ExitStack() as ctx:
        tc = ctx.enter_context(tile.TileContext(nc))
        wpool = ctx.enter_context(tc.tile_pool(name="w", bufs=1))
        xpool = ctx.enter_context(tc.tile_pool(name="xh", bufs=3))
        cpool = ctx.enter_context(tc.tile_pool(name="cin", bufs=6))
        gpool = ctx.enter_context(tc.tile_pool(name="gates", bufs=3))
        opool = ctx.enter_context(tc.tile_pool(name="outs", bufs=3))
        pspool = ctx.enter_context(tc.tile_pool(name="ps", bufs=2, space="PSUM"))

        # PE HAM warmup (see module docstring).
        wu = wpool.tile([P, P], BF16, tag="wu", name="wu")
        nc.vector.memset(wu[:], 0.0)
        wu_ps = pspool.tile([P, JW], F32, tag="ps0", name="wu_ps")
        for _ in range(NWU):
            nc.tensor.matmul(wu_ps[:, :P], wu[:], wu[:], start=True, stop=True)

        # Weight tiles: r0 as 4 quarter tiles (2 k-chunks each), r1-3 as
        # 2 half tiles (4 k-chunks each).  wsl[r] -> list of
        # (tile, k_chunks_per_tile) so MMs can slice uniformly.
        bias_t = wpool.tile([P, NM], F32, tag="bias", name="bias")
        wq = [wpool.tile([P, 2, JW], BF16, tag=f"wq{q}", name=f"wq{q}")
              for q in range(4)]
        wh = {r: [wpool.tile([P, 4, JW], BF16, tag=f"w{r}{h}", name=f"w{r}{h}")
                  for h in range(2)] for r in range(1, NR)}

        def wslice(r, k, g):
            if r == 0:
                return wq[k // 2][:, k % 2, g * P:(g + 1) * P]
            t = wh[r][k // 4]
            return t[:, k % 4, g * P:(g + 1) * P]

        xh_tiles = {}   # nb0: per-k [P,JW]; nb>=1: per-nb [P,NK,JW]

        def xh_rhs(nb, k, s=slice(None)):
            if nb == 0:
                return xh_tiles[(0, k)][:, s]
            return xh_tiles[nb][:, k, s]

        c_tiles = {}

        def load_c(nb, r):
            ct = cpool.tile([P, JW], F32, tag="c")
            nc.scalar.dma_start(ct[:], c_in[nb, r])
            c_tiles[(nb, r)] = ct

        # Startup DMAs ride BOTH hardware queues so the warm-rate k-outer
        # consumption of r0 never starves: weights on the sync queue,
        # xh0 chunks + bias + c tiles on the scalar queue.  Each queue's
        # entries are in consumption-deadline order and DRAM-contiguous.
        for q in range(4):
            nc.sync.dma_start(wq[q][:], wq_in[q])
        for k in range(NK):
            xt = xpool.tile([P, JW], BF16, tag=f"xh{k}", name=f"xh{k}")
            nc.scalar.dma_start(xt[:], xh0_in[k])
            xh_tiles[(0, k)] = xt
        nc.scalar.dma_start(bias_t[:], bias_in[:])
        nc.sync.dma_start(wh[1][0][:], wh_in[0, 0])
        load_c(0, 0)
        nc.sync.dma_start(wh[1][1][:], wh_in[0, 1])
        nc.sync.dma_start(wh[2][0][:], wh_in[1, 0])
        nc.sync.dma_start(wh[2][1][:], wh_in[1, 1])
        nc.sync.dma_start(wh[3][0][:], wh_in[2, 0])
        nc.sync.dma_start(wh[3][1][:], wh_in[2, 1])
        # c[0,1..3] triggers are deferred into the nb0 r-loop: they are not
        # consumed until ~26-40us, and issuing them here would steal early
        # DMA bandwidth from wh10 (the r1 weight stall).

        for nb in range(nbn):
            if nb > 0:
                xt = xpool.tile([P, NK, JW], BF16, tag="xhb", name="xhb")
                nc.sync.dma_start(xt[:], xhb_in[nb - 1])
                xh_tiles[nb] = xt
                for r in range(NR):
                    load_c(nb, r)
            last_nb = nb == nbn - 1
            for r in range(NR):
                if nb == 0 and r >= 1:
                    load_c(0, r)
                ct = c_tiles[(nb, r)]
                last_grp = last_nb and r == NR - 1
                if last_grp:
                    break
                ps = [
                    pspool.tile([P, JW], F32, tag=f"ps{g}", name=f"ps{g}")
                    for g in range(4)
                ]
                if nb == 0:
                    # k-outer/g-inner: consume weight chunks as they land.
                    for k in range(NK):
                        for g in range(4):
                            nc.tensor.matmul(
                                ps[g][:], wslice(r, k, g), xh_rhs(nb, k),
                                start=(k == 0), stop=(k == NK - 1),
                            )
                else:
                    # g-outer/k-inner: per-gate groups finish staggered so
                    # the epilogue pipelines under the next gate's MMs.
                    for g in range(4):
                        for k in range(NK):
                            nc.tensor.matmul(
                                ps[g][:], wslice(r, k, g), xh_rhs(nb, k),
                                start=(k == 0), stop=(k == NK - 1),
                            )
                it = gpool.tile([P, JW], F32, tag="i")
                gt = gpool.tile([P, JW], F32, tag="g")
                ft = gpool.tile([P, JW], F32, tag="f")
                ot = gpool.tile([P, JW], F32, tag="o")
                t1 = gpool.tile([P, JW], F32, tag="t1")
                t2 = gpool.tile([P, JW], F32, tag="t2")
                cn = opool.tile([P, JW], F32, tag="cn")
                tch = gpool.tile([P, JW], F32, tag="tch")
                hn = opool.tile([P, JW], F32, tag="hn")
                nc.scalar.activation(
                    it[:], ps[0][:], AF.Sigmoid, bias=bias_t[:, 0 + r : 1 + r]
                )
                nc.scalar.activation(
                    gt[:], ps[1][:], AF.Tanh, bias=bias_t[:, NR + r : NR + r + 1]
                )
                nc.scalar.activation(
                    ft[:], ps[2][:], AF.Sigmoid,
                    bias=bias_t[:, 2 * NR + r : 2 * NR + r + 1],
                )
                nc.scalar.activation(
                    ot[:], ps[3][:], AF.Sigmoid,
                    bias=bias_t[:, 3 * NR + r : 3 * NR + r + 1],
                )
                nc.vector.tensor_mul(t1[:], it[:], gt[:])
                nc.vector.tensor_mul(t2[:], ft[:], ct[:])
                nc.vector.tensor_add(cn[:], t1[:], t2[:])
                nc.scalar.activation(tch[:], cn[:], AF.Tanh)
                nc.vector.tensor_mul(hn[:], ot[:], tch[:])
                if last_nb:
                    nc.sync.dma_start(c_out[nb, r], cn[:])
                    nc.sync.dma_start(h_out[nb, r], hn[:])
                else:
                    nc.gpsimd.dma_start(c_out[nb, r], cn[:])
                    nc.gpsimd.dma_start(h_out[nb, r], hn[:])

        # Final tile (nb=nbn-1, r=NR-1): two sequential N=256 PSUM groups.
        # The first half's epilogue runs under the second half's matmuls,
        # so the post-last-matmul critical path is one half-width chain.
        nb, r = nbn - 1, NR - 1
        ct = c_tiles[(nb, r)]
        it = gpool.tile([P, JW], F32, tag="i")
        gt = gpool.tile([P, JW], F32, tag="g")
        ft = gpool.tile([P, JW], F32, tag="f")
        ot = gpool.tile([P, JW], F32, tag="o")
        t1 = gpool.tile([P, JW], F32, tag="t1")
        t2 = gpool.tile([P, JW], F32, tag="t2")
        cn = opool.tile([P, JW], F32, tag="cn")
        tch = gpool.tile([P, JW], F32, tag="tch")
        hn = opool.tile([P, JW], F32, tag="hn")
        for s in (slice(0, H2), slice(H2, JW)):
            ps = [
                pspool.tile([P, JW], F32, tag=f"ps{g}", name=f"ps{g}")
                for g in range(4)
            ]
            for g in range(4):
                for k in range(NK):
                    nc.tensor.matmul(
                        ps[g][:, :H2], wslice(r, k, g), xh_rhs(nb, k, s),
                        start=(k == 0), stop=(k == NK - 1),
                    )
            nc.scalar.activation(
                it[:, s], ps[0][:, :H2], AF.Sigmoid, bias=bias_t[:, r:r + 1]
            )
            nc.scalar.activation(
                gt[:, s], ps[1][:, :H2], AF.Tanh,
                bias=bias_t[:, NR + r:NR + r + 1],
            )
            nc.scalar.activation(
                ft[:, s], ps[2][:, :H2], AF.Sigmoid,
                bias=bias_t[:, 2 * NR + r:2 * NR + r + 1],
            )
            nc.scalar.activation(
                ot[:, s], ps[3][:, :H2], AF.Sigmoid,
                bias=bias_t[:, 3 * NR + r:3 * NR + r + 1],
            )
            nc.vector.tensor_mul(t1[:, s], it[:, s], gt[:, s])
            nc.vector.tensor_mul(t2[:, s], ft[:, s], ct[:, s])
            nc.vector.tensor_add(cn[:, s], t1[:, s], t2[:, s])
            nc.sync.dma_start(c_out[nb, r][:, s], cn[:, s])
            nc.scalar.activation(tch[:, s], cn[:, s], AF.Tanh)
            nc.vector.tensor_mul(hn[:, s], ot[:, s], tch[:, s])
            nc.sync.dma_start(h_out[nb, r][:, s], hn[:, s])
    nc.compile()
    return nc


def prep_shared(Wxi, Wxg, Wxf, Wxo, Whi, Whg, Whf, Who, bias_sum):
    """wq_in [4,P,2,JW] / wh_in [NR-1,2,P,4,JW] bf16, bias_in [P,NM] f32."""
    Wx = np.concatenate([Wxi, Wxg, Wxf, Wxo], axis=0)  # [4H, IN]
    Wh = np.concatenate([Whi, Whg, Whf, Who], axis=0)  # [4H, HID]
    WT = np.concatenate([Wx.T, Wh.T], axis=0).astype(BF16_NP)  # [K=1024, 4H]
    wh_arr = np.empty((NR - 1, 2, P, 4, JW), BF16_NP)
    for r in range(NR):
        cols = np.concatenate(
            [WT[:, (g * NR + r) * P:(g * NR + r + 1) * P] for g in range(4)],
            axis=1,
        )  # [K, 4*128]
        if r == 0:
            wq_arr = np.ascontiguousarray(
                cols.reshape(4, 2, P, JW).transpose(0, 2, 1, 3)
            )
        else:
            wh_arr[r - 1] = cols.reshape(2, 4, P, JW).transpose(0, 2, 1, 3)
    wh_arr = np.ascontiguousarray(wh_arr)
    bias_arr = np.ascontiguousarray(
        bias_sum.reshape(NM, P).T.astype(np.float32)
    )
    return wq_arr, wh_arr, bias_arr


def prep_core(x_s, h_s, c_s):
    """Per-core xh0_in [NK,P,JW] + xhb_in [nb-1,P,NK,JW] bf16, c_in f32."""
    bl = x_s.shape[0]
    nbn = bl // JW
    xhT = np.concatenate([x_s, h_s], axis=1).T  # [K=1024, bl]
    xh = xhT.reshape(NK, P, nbn, JW).astype(BF16_NP)
    xh0_arr = np.ascontiguousarray(xh[:, :, 0, :])
    out = {"xh0_in": xh0_arr}
    if nbn > 1:
        out["xhb_in"] = np.ascontiguousarray(
            xh[:, :, 1:, :].transpose(2, 1, 0, 3)
        )
    cT = c_s.T  # [HID, bl]
    out["c_in"] = np.ascontiguousarray(
        cT.reshape(NR, P, nbn, JW).transpose(2, 0, 1, 3).astype(np.float32)
    )
    return out


def post_core(arr):
    """[nb,NR,P,JW] -> [bl, HID]"""
    arr = np.asarray(arr)
    nbn = arr.size // (NR * P * JW)
    arr = arr.reshape(nbn, NR, P, JW)
    return arr.transpose(0, 3, 1, 2).reshape(nbn * JW, HID)


_NC_CACHE = {}


def _get_nc(bl=BL):
    if bl not in _NC_CACHE:
        _NC_CACHE[bl] = build_nc(bl)
    return _NC_CACHE[bl]


def make_in_maps(x, h, c, Wxi, bxi, Wxo, bxo, Wxf, bxf, Wxg, bxg,
                 Whi, bhi, Who, bho, Whf, bhf, Whg, bhg, ncores=NCORES):
    bias_sum = np.concatenate(
        [bxi + bhi, bxg + bhg, bxf + bhf, bxo + bho], axis=0
    ).astype(np.float32)
    wq_arr, wh_arr, bias_arr = prep_shared(
        Wxi, Wxg, Wxf, Wxo, Whi, Whg, Whf, Who, bias_sum
    )
    bl = x.shape[0] // ncores
    in_maps = []
    for i in range(ncores):
        s = slice(i * bl, (i + 1) * bl)
        core = prep_core(
            np.asarray(x[s], np.float32),
            np.asarray(h[s], np.float32),
            np.asarray(c[s], np.float32),
        )
        in_maps.append(
            {"wq_in": wq_arr, "wh_in": wh_arr, "bias_in": bias_arr, **core}
        )
    return in_maps


def kernel(x, h, c, Wxi, bxi, Wxo, bxo, Wxf, bxf, Wxg, bxg,
           Whi, bhi, Who, bho, Whf, bhf, Whg, bhg):
    args = dict(
        x=np.asarray(x, np.float32), h=np.asarray(h, np.float32),
        c=np.asarray(c, np.float32),
        Wxi=np.asarray(Wxi, np.float32), bxi=np.asarray(bxi, np.float32),
        Wxo=np.asarray(Wxo, np.float32), bxo=np.asarray(bxo, np.float32),
        Wxf=np.asarray(Wxf, np.float32), bxf=np.asarray(bxf, np.float32),
        Wxg=np.asarray(Wxg, np.float32), bxg=np.asarray(bxg, np.float32),
        Whi=np.asarray(Whi, np.float32), bhi=np.asarray(bhi, np.float32),
        Who=np.asarray(Who, np.float32), bho=np.asarray(bho, np.float32),
        Whf=np.asarray(Whf, np.float32), bhf=np.asarray(bhf, np.float32),
        Whg=np.asarray(Whg, np.float32), bhg=np.asarray(bhg, np.float32),
    )
    in_maps = make_in_maps(**args)
    nc = _get_nc(BL)
    res = run_bass_kernel_spmd(nc, in_maps, core_ids=list(range(NCORES)))
    h_new = np.empty((B_FULL, HID), np.float32)
    c_new = np.empty((B_FULL, HID), np.float32)
    for i in range(NCORES):
        s = slice(i * BL, (i + 1) * BL)
        h_new[s] = post_core(res.results[i]["h_out"])
        c_new[s] = post_core(res.results[i]["c_out"])
    return (h_new, c_new)


# revision 28
# speedup vs baseline: 1.0101x; 1.0101x over previous
"""LSTMCell (B=16384, IN=HID=512) on 8 TRN2 NeuronCores.

Strategy: data-parallel over batch (2048 rows/core), weights replicated.
Host pre-packs operands so the device kernel needs zero transposes:
  - GEMM computed as gates.T = W_cat.T @ [x;h].T  (K=1024 on partitions)
  - x/h/W cast to bf16 on host (fp32 PSUM accumulation on PE)
  - c / outputs stay fp32

v3 structure (baseline 135.7us -> v2 132.3us):
  - Weights repacked per output row-block r ([NR, P, NK, JW] DRAM layout,
    per-partition-contiguous lines).  r0's weights arrive as four 256KB
    quarter tiles so the very first matmul only needs 384KB of input;
    r1-r3 use 512KB half tiles.  All input DMAs ride the sync HW queue
    ordered by consumption deadline (bias first - it gates the first ACT).
  - nb0 runs k-outer/g-inner so weight chunks are consumed as they land;
    nb1-3 run g-outer/k-inner so the per-gate epilogue pipelines.
  - nb1-3 xh arrives as one 1MB DMA per nb (8KB/partition lines).
  - Output DMAs ride the gpsimd SW-DGE queue (never block inputs);
    the final nb's outputs ride sync (empty by then).
  - Last tile runs as two sequential N=256 PSUM groups so its epilogue
    overlaps the second half's matmuls; ACT(o) is queued before tanh.
  - 34 N=128 warmup matmuls keep the PE HAM busy-window alive from the
    first possible instruction (~7.6us) until data lands (~10.6us) so
    real matmuls run at 2.4GHz immediately.
"""

import sys

sys.path.insert(0, "/opt/trn_rl_repo")

from contextlib import ExitStack

import ml_dtypes
import numpy as np

import concourse.bass as bass  # noqa: F401  (bass types used via bacc/mybir)
import concourse.mybir as mybir
import concourse.tile as tile
from concourse import bacc
from concourse.bass_utils import run_bass_kernel_spmd

B_FULL, IN, HID = 16384, 512, 512
NCORES = 8
BL = B_FULL // NCORES  # 2048 batch rows per core
JW = 512               # batch columns per chunk (matmul free dim)
P = 128
H2 = JW // 2

BF16 = mybir.dt.bfloat16
F32 = mybir.dt.float32
AF = mybir.ActivationFunctionType
BF16_NP = ml_dtypes.bfloat16

NK = (IN + HID) // P   # 8  k-chunks of the contraction dim
NR = HID // P          # 4  row-blocks of H per gate
NM = 4 * HID // P      # 16 gate-row blocks total (i,g,f,o order)
NWU = 46               # warmup matmuls (N=128, ~107ns apiece cold); sized so
                       # they end ~12.1us, when the first input DMA semaphore
                       # fires (bytes land ~10.2us + ~1.7us completion latency)


def build_nc(bl=BL):
    """Build the single-core Bass program (SPMD-replicated across cores)."""
    nbn = bl // JW
    nc = bacc.Bacc("TRN2", target_bir_lowering=False, debug=False)

    # nb0's xh: k-major contiguous 128KB chunks (fine-grained startup);
    # nb>=1: partition-major 1MB blocks (8KB/partition lines, one DMA each).
    xh0_in = nc.dram_tensor("xh0_in", [NK, P, JW], BF16, kind="ExternalInput")
    if nbn > 1:
        xhb_in = nc.dram_tensor("xhb_in", [nbn - 1, P, NK, JW], BF16,
                                kind="ExternalInput")
    # r0 weights as 4 contiguous 256KB quarter chunks; r1-3 as contiguous
    # 512KB half chunks.
    wq_in = nc.dram_tensor("wq_in", [4, P, 2, JW], BF16, kind="ExternalInput")
    wh_in = nc.dram_tensor("wh_in", [NR - 1, 2, P, 4, JW], BF16,
                           kind="ExternalInput")
    bias_in = nc.dram_tensor("bias_in", [P, NM], F32, kind="ExternalInput")
    c_in = nc.dram_tensor("c_in", [nbn, NR, P, JW], F32, kind="ExternalInput")
    h_out = nc.dram_tensor("h_out", [nbn, NR, P, JW], F32, kind="ExternalOutput")
    c_out = nc.dram_tensor("c_out", [nbn, NR, P, JW], F32, kind="ExternalOutput")

    with ExitStack() as ctx:
        tc = ctx.enter_context(tile.TileContext(nc))
        wpool = ctx.enter_context(tc.tile_pool(name="w", bufs=1))
        xpool = ctx.enter_context(tc.tile_pool(name="xh", bufs=3))
        cpool = ctx.enter_context(tc.tile_pool(name="cin", bufs=6))
        gpool = ctx.enter_context(tc.tile_pool(name="gates", bufs=3))
        opool = ctx.enter_context(tc.tile_pool(name="outs", bufs=3))
        pspool = ctx.enter_context(tc.tile_pool(name="ps", bufs=2, space="PSUM"))

        # PE HAM warmup (see module docstring).
        wu = wpool.tile([P, P], BF16, tag="wu", name="wu")
        nc.vector.memset(wu[:], 0.0)
        wu_ps = pspool.tile([P, JW], F32, tag="ps0", name="wu_ps")
        for _ in range(NWU):
            nc.tensor.matmul(wu_ps[:, :P], wu[:], wu[:], start=True, stop=True)

        # Weight tiles: r0 as 4 quarter tiles (2 k-chunks each), r1-3 as
        # 2 half tiles (4 k-chunks each).  wsl[r] -> list of
        # (tile, k_chunks_per_tile) so MMs can slice uniformly.
        bias_t = wpool.tile([P, NM], F32, tag="bias", name="bias")
        wq = [wpool.tile([P, 2, JW], BF16, tag=f"wq{q}", name=f"wq{q}")
              for q in range(4)]
        wh = {r: [wpool.tile([P, 4, JW], BF16, tag=f"w{r}{h}", name=f"w{r}{h}")
                  for h in range(2)] for r in range(1, NR)}

        def wslice(r, k, g):
            if r == 0:
                return wq[k // 2][:, k % 2, g * P:(g + 1) * P]
            t = wh[r][k // 4]
            return t[:, k % 4, g * P:(g + 1) * P]

        xh_tiles = {}   # nb0: per-k [P,JW]; nb>=1: per-nb [P,NK,JW]

        def xh_rhs(nb, k, s=slice(None)):
            if nb == 0:
                return xh_tiles[(0, k)][:, s]
            return xh_tiles[nb][:, k, s]

        c_tiles = {}

        def load_c(nb, r):
            ct = cpool.tile([P, JW], F32, tag="c")
            nc.scalar.dma_start(ct[:], c_in[nb, r])
            c_tiles[(nb, r)] = ct

        # Startup DMAs ride BOTH hardware queues so the warm-rate k-outer
        # consumption of r0 never starves: weights on the sync queue,
        # xh0 chunks + bias + c tiles on the scalar queue.  Each queue's
        # entries are in consumption-deadline order and DRAM-contiguous.
        for q in range(4):
            nc.sync.dma_start(wq[q][:], wq_in[q])
        for k in range(NK):
            xt = xpool.tile([P, JW], BF16, tag=f"xh{k}", name=f"xh{k}")
            nc.scalar.dma_start(xt[:], xh0_in[k])
            xh_tiles[(0, k)] = xt
        nc.scalar.dma_start(bias_t[:], bias_in[:])
        nc.sync.dma_start(wh[1][0][:], wh_in[0, 0])
        nc.sync.dma_start(wh[1][1][:], wh_in[0, 1])
        nc.sync.dma_start(wh[2][0][:], wh_in[1, 0])
        nc.sync.dma_start(wh[2][1][:], wh_in[1, 1])
        nc.sync.dma_start(wh[3][0][:], wh_in[2, 0])
        nc.sync.dma_start(wh[3][1][:], wh_in[2, 1])
        # c[0,1..3] triggers are deferred into the nb0 r-loop: they are not
        # consumed until ~26-40us, and issuing them here would steal early
        # DMA bandwidth from wh10 (the r1 weight stall).

        for nb in range(nbn):
            if nb > 0:
                xt = xpool.tile([P, NK, JW], BF16, tag="xhb", name="xhb")
                nc.sync.dma_start(xt[:], xhb_in[nb - 1])
                xh_tiles[nb] = xt
                for r in range(NR):
                    load_c(nb, r)
            last_nb = nb == nbn - 1
            for r in range(NR):
                if nb == 0 and r >= 1:
                    load_c(0, r)
                last_grp = last_nb and r == NR - 1
                if last_grp:
                    break
                ps = [
                    pspool.tile([P, JW], F32, tag=f"ps{g}", name=f"ps{g}")
                    for g in range(4)
                ]
                if nb == 0:
                    # k-outer/g-inner: consume weight chunks as they land.
                    for k in range(NK):
                        for g in range(4):
                            nc.tensor.matmul(
                                ps[g][:], wslice(r, k, g), xh_rhs(nb, k),
                                start=(k == 0), stop=(k == NK - 1),
                            )
                else:
                    # g-outer/k-inner: per-gate groups finish staggered so
                    # the epilogue pipelines under the next gate's MMs.
                    for g in range(4):
                        for k in range(NK):
                            nc.tensor.matmul(
                                ps[g][:], wslice(r, k, g), xh_rhs(nb, k),
                                start=(k == 0), stop=(k == NK - 1),
                            )
                it = gpool.tile([P, JW], F32, tag="i")
                gt = gpool.tile([P, JW], F32, tag="g")
                ft = gpool.tile([P, JW], F32, tag="f")
                ot = gpool.tile([P, JW], F32, tag="o")
                t1 = gpool.tile([P, JW], F32, tag="t1")
                t2 = gpool.tile([P, JW], F32, tag="t2")
                cn = opool.tile([P, JW], F32, tag="cn")
                tch = gpool.tile([P, JW], F32, tag="tch")
                hn = opool.tile([P, JW], F32, tag="hn")
                nc.scalar.activation(
                    it[:], ps[0][:], AF.Sigmoid, bias=bias_t[:, 0 + r : 1 + r]
                )
                if nb == 0 and r == 0:
                    # c[0,0] trigger deferred to here (needed only by t2,
                    # ~1.5us later): keeps its 256KB out of the startup
                    # window that feeds r0/r1 weights at warm-MM rate.
                    load_c(0, 0)
                nc.scalar.activation(
                    gt[:], ps[1][:], AF.Tanh, bias=bias_t[:, NR + r : NR + r + 1]
                )
                nc.scalar.activation(
                    ft[:], ps[2][:], AF.Sigmoid,
                    bias=bias_t[:, 2 * NR + r : 2 * NR + r + 1],
                )
                nc.scalar.activation(
                    ot[:], ps[3][:], AF.Sigmoid,
                    bias=bias_t[:, 3 * NR + r : 3 * NR + r + 1],
                )
                ct = c_tiles[(nb, r)]
                nc.vector.tensor_mul(t1[:], it[:], gt[:])
                nc.vector.tensor_mul(t2[:], ft[:], ct[:])
                nc.vector.tensor_add(cn[:], t1[:], t2[:])
                nc.scalar.activation(tch[:], cn[:], AF.Tanh)
                nc.vector.tensor_mul(hn[:], ot[:], tch[:])
                if last_nb:
                    nc.sync.dma_start(c_out[nb, r], cn[:])
                    nc.sync.dma_start(h_out[nb, r], hn[:])
                else:
                    nc.gpsimd.dma_start(c_out[nb, r], cn[:])
                    nc.gpsimd.dma_start(h_out[nb, r], hn[:])

        # Final tile (nb=nbn-1, r=NR-1): two sequential N=256 PSUM groups.
        # The first half's epilogue runs under the second half's matmuls,
        # so the post-last-matmul critical path is one half-width chain.
        nb, r = nbn - 1, NR - 1
        ct = c_tiles[(nb, r)]
        it = gpool.tile([P, JW], F32, tag="i")
        gt = gpool.tile([P, JW], F32, tag="g")
        ft = gpool.tile([P, JW], F32, tag="f")
        ot = gpool.tile([P, JW], F32, tag="o")
        t1 = gpool.tile([P, JW], F32, tag="t1")
        t2 = gpool.tile([P, JW], F32, tag="t2")
        cn = opool.tile([P, JW], F32, tag="cn")
        tch = gpool.tile([P, JW], F32, tag="tch")
        hn = opool.tile([P, JW], F32, tag="hn")
        for s in (slice(0, H2), slice(H2, JW)):
            ps = [
                pspool.tile([P, JW], F32, tag=f"ps{g}", name=f"ps{g}")
                for g in range(4)
            ]
            for g in range(4):
                for k in range(NK):
                    nc.tensor.matmul(
                        ps[g][:, :H2], wslice(r, k, g), xh_rhs(nb, k, s),
                        start=(k == 0), stop=(k == NK - 1),
                    )
            nc.scalar.activation(
                it[:, s], ps[0][:, :H2], AF.Sigmoid, bias=bias_t[:, r:r + 1]
            )
            nc.scalar.activation(
                gt[:, s], ps[1][:, :H2], AF.Tanh,
                bias=bias_t[:, NR + r:NR + r + 1],
            )
            nc.scalar.activation(
                ft[:, s], ps[2][:, :H2], AF.Sigmoid,
                bias=bias_t[:, 2 * NR + r:2 * NR + r + 1],
            )
            nc.scalar.activation(
                ot[:, s], ps[3][:, :H2], AF.Sigmoid,
                bias=bias_t[:, 3 * NR + r:3 * NR + r + 1],
            )
            nc.vector.tensor_mul(t1[:, s], it[:, s], gt[:, s])
            nc.vector.tensor_mul(t2[:, s], ft[:, s], ct[:, s])
            nc.vector.tensor_add(cn[:, s], t1[:, s], t2[:, s])
            nc.sync.dma_start(c_out[nb, r][:, s], cn[:, s])
            nc.scalar.activation(tch[:, s], cn[:, s], AF.Tanh)
            nc.vector.tensor_mul(hn[:, s], ot[:, s], tch[:, s])
            nc.sync.dma_start(h_out[nb, r][:, s], hn[:, s])
    nc.compile()
    return nc


def prep_shared(Wxi, Wxg, Wxf, Wxo, Whi, Whg, Whf, Who, bias_sum):
    """wq_in [4,P,2,JW] / wh_in [NR-1,2,P,4,JW] bf16, bias_in [P,NM] f32."""
    Wx = np.concatenate([Wxi, Wxg, Wxf, Wxo], axis=0)  # [4H, IN]
    Wh = np.concatenate([Whi, Whg, Whf, Who], axis=0)  # [4H, HID]
    WT = np.concatenate([Wx.T, Wh.T], axis=0).astype(BF16_NP)  # [K=1024, 4H]
    wh_arr = np.empty((NR - 1, 2, P, 4, JW), BF16_NP)
    for r in range(NR):
        cols = np.concatenate(
            [WT[:, (g * NR + r) * P:(g * NR + r + 1) * P] for g in range(4)],
            axis=1,
        )  # [K, 4*128]
        if r == 0:
            wq_arr = np.ascontiguousarray(
                cols.reshape(4, 2, P, JW).transpose(0, 2, 1, 3)
            )
        else:
            wh_arr[r - 1] = cols.reshape(2, 4, P, JW).transpose(0, 2, 1, 3)
    wh_arr = np.ascontiguousarray(wh_arr)
    bias_arr = np.ascontiguousarray(
        bias_sum.reshape(NM, P).T.astype(np.float32)
    )
    return wq_arr, wh_arr, bias_arr


def prep_core(x_s, h_s, c_s):
    """Per-core xh0_in [NK,P,JW] + xhb_in [nb-1,P,NK,JW] bf16, c_in f32."""
    bl = x_s.shape[0]
    nbn = bl // JW
    xhT = np.concatenate([x_s, h_s], axis=1).T  # [K=1024, bl]
    xh = xhT.reshape(NK, P, nbn, JW).astype(BF16_NP)
    xh0_arr = np.ascontiguousarray(xh[:, :, 0, :])
    out = {"xh0_in": xh0_arr}
    if nbn > 1:
        out["xhb_in"] = np.ascontiguousarray(
            xh[:, :, 1:, :].transpose(2, 1, 0, 3)
        )
    cT = c_s.T  # [HID, bl]
    out["c_in"] = np.ascontiguousarray(
        cT.reshape(NR, P, nbn, JW).transpose(2, 0, 1, 3).astype(np.float32)
    )
    return out


def post_core(arr):
    """[nb,NR,P,JW] -> [bl, HID]"""
    arr = np.asarray(arr)
    nbn = arr.size // (NR * P * JW)
    arr = arr.reshape(nbn, NR, P, JW)
    return arr.transpose(0, 3, 1, 2).reshape(nbn * JW, HID)


_NC_CACHE = {}


def _get_nc(bl=BL):
    if bl not in _NC_CACHE:
        _NC_CACHE[bl] = build_nc(bl)
    return _NC_CACHE[bl]


def make_in_maps(x, h, c, Wxi, bxi, Wxo, bxo, Wxf, bxf, Wxg, bxg,
                 Whi, bhi, Who, bho, Whf, bhf, Whg, bhg, ncores=NCORES):
    bias_sum = np.concatenate(
        [bxi + bhi, bxg + bhg, bxf + bhf, bxo + bho], axis=0
    ).astype(np.float32)
    wq_arr, wh_arr, bias_arr = prep_shared(
        Wxi, Wxg, Wxf, Wxo, Whi, Whg, Whf, Who, bias_sum
    )
    bl = x.shape[0] // ncores
    in_maps = []
    for i in range(ncores):
        s = slice(i * bl, (i + 1) * bl)
        core = prep_core(
            np.asarray(x[s], np.float32),
            np.asarray(h[s], np.float32),
            np.asarray(c[s], np.float32),
        )
        in_maps.append(
            {"wq_in": wq_arr, "wh_in": wh_arr, "bias_in": bias_arr, **core}
        )
    return in_maps


def kernel(x, h, c, Wxi, bxi, Wxo, bxo, Wxf, bxf, Wxg, bxg,
           Whi, bhi, Who, bho, Whf, bhf, Whg, bhg):
    args = dict(
        x=np.asarray(x, np.float32), h=np.asarray(h, np.float32),
        c=np.asarray(c, np.float32),
        Wxi=np.asarray(Wxi, np.float32), bxi=np.asarray(bxi, np.float32),
        Wxo=np.asarray(Wxo, np.float32), bxo=np.asarray(bxo, np.float32),
        Wxf=np.asarray(Wxf, np.float32), bxf=np.asarray(bxf, np.float32),
        Wxg=np.asarray(Wxg, np.float32), bxg=np.asarray(bxg, np.float32),
        Whi=np.asarray(Whi, np.float32), bhi=np.asarray(bhi, np.float32),
        Who=np.asarray(Who, np.float32), bho=np.asarray(bho, np.float32),
        Whf=np.asarray(Whf, np.float32), bhf=np.asarray(bhf, np.float32),
        Whg=np.asarray(Whg, np.float32), bhg=np.asarray(bhg, np.float32),
    )
    in_maps = make_in_maps(**args)
    nc = _get_nc(BL)
    res = run_bass_kernel_spmd(nc, in_maps, core_ids=list(range(NCORES)))
    h_new = np.empty((B_FULL, HID), np.float32)
    c_new = np.empty((B_FULL, HID), np.float32)
    for i in range(NCORES):
        s = slice(i * BL, (i + 1) * BL)
        h_new[s] = post_core(res.results[i]["h_out"])
        c_new[s] = post_core(res.results[i]["c_out"])
    return (h_new, c_new)
